# revision 5
# baseline (speedup 1.0000x reference)
"""MoE (8 routed experts, top-2, + shared expert) on 8 NeuronCores.

Strategy: data-parallel over tokens (1024 tokens/core), gate + all expert
weights replicated. The shared expert (hidden 4096) is split into two
H=2048 halves so the kernel is a uniform loop over 10 "virtual experts".
Dense formulation: every expert processes every token, scaled by the
(zero for unrouted) renormalized top-2 combine weight, fused into the
PSUM eviction. Gate runs in fp32 so routing decisions match the
reference; expert matmuls run in bf16 with fp32 accumulation.
"""

import numpy as np
import ml_dtypes

import concourse.bacc as bacc
import concourse.tile as tile
import concourse.mybir as mybir
from concourse.bass_utils import run_bass_kernel_spmd

BF16 = ml_dtypes.bfloat16
F32 = mybir.dt.float32
BF = mybir.dt.bfloat16
AF = mybir.ActivationFunctionType
OP = mybir.AluOpType

P = 128


class Cfg:
    def __init__(self, D=1024, H=2048, E=8, n_sh=2, T=1024, n_cores=8):
        self.D, self.H, self.E, self.n_sh, self.T = D, H, E, n_sh, T
        self.NV = E + n_sh          # virtual experts
        self.HS = n_sh * H          # shared hidden
        self.KD = D // P            # K chunks over D
        self.HCN = H // P           # h chunks over H
        self.TT = T // P            # token 128-tiles
        self.DT = (D + 511) // 512  # output d 512-tiles
        self.FT = (T + 511) // 512  # layer-1 free 512-tiles
        self.n_cores = n_cores


def build_nc(cfg: Cfg):
    D, H, E, NV, T = cfg.D, cfg.H, cfg.E, cfg.NV, cfg.T
    KD, HCN, TT, DT, FT = cfg.KD, cfg.HCN, cfg.TT, cfg.DT, cfg.FT

    nc = bacc.Bacc("TRN2", target_bir_lowering=False)

    xT = nc.dram_tensor("xT", [P, KD, T], F32, kind="ExternalInput")
    w1t = nc.dram_tensor("w1t", [NV, HCN, P, KD, P], BF, kind="ExternalInput")
    w3t = nc.dram_tensor("w3t", [NV, HCN, P, KD, P], BF, kind="ExternalInput")
    w2t = nc.dram_tensor("w2t", [NV, P, HCN, D], BF, kind="ExternalInput")
    b1a = nc.dram_tensor("b1a", [NV, P, HCN], F32, kind="ExternalInput")
    b3a = nc.dram_tensor("b3a", [NV, P, HCN], F32, kind="ExternalInput")
    b2r = nc.dram_tensor("b2r", [1, NV, D], BF, kind="ExternalInput")
    gwt = nc.dram_tensor("gwt", [P, KD, E], F32, kind="ExternalInput")
    gb = nc.dram_tensor("gb", [1, E], F32, kind="ExternalInput")
    ones1 = nc.dram_tensor("ones1", [1, P], BF, kind="ExternalInput")
    y = nc.dram_tensor("y", [P, TT, D], F32, kind="ExternalOutput")

    with tile.TileContext(nc) as tc:
        with (
            tc.tile_pool(name="const1", bufs=1) as const1,
            tc.tile_pool(name="gchunk", bufs=2) as gchunk,
            tc.tile_pool(name="gtmp", bufs=4) as gtmp,
            tc.tile_pool(name="w1s", bufs=3) as w1s,
            tc.tile_pool(name="b13", bufs=2) as b13,
            tc.tile_pool(name="w2s", bufs=2) as w2s,
            tc.tile_pool(name="hpool", bufs=1) as hpool,
            tc.tile_pool(name="s1p", bufs=3) as s1p,
            tc.tile_pool(name="ps_l1", bufs=2, space="PSUM") as ps_l1,
            tc.tile_pool(name="ps_y", bufs=2, space="PSUM") as ps_y,
            tc.tile_pool(name="ps_g", bufs=2, space="PSUM") as ps_g,
        ):
            # ---- resident constants ----
            xTb = const1.tile([P, KD, T], BF)
            cw = const1.tile([P, TT, NV], F32)
            yacc = const1.tile([P, TT, D], F32)
            b2r_sb = const1.tile([1, NV, D], BF)
            ones_sb = const1.tile([1, P], BF)
            gwt_sb = const1.tile([P, KD, E], F32)
            gb_sb = const1.tile([1, E], F32)
            zerob = const1.tile([P, 1], F32)
            onesf = const1.tile([1, P], F32)

            nc.sync.dma_start(out=b2r_sb[:], in_=b2r[:])
            nc.sync.dma_start(out=ones_sb[:], in_=ones1[:])
            nc.sync.dma_start(out=gwt_sb[:], in_=gwt[:])
            nc.sync.dma_start(out=gb_sb[:], in_=gb[:])
            nc.vector.memset(zerob[:], 0.0)
            nc.vector.memset(onesf[:], 1.0)

            # ---- gate + bf16 cast of activations, per 128-token tile ----
            for m in range(TT):
                xchunk = gchunk.tile([P, KD, P], F32)
                nc.sync.dma_start(out=xchunk[:], in_=xT[:, :, m * P:(m + 1) * P])
                nc.vector.tensor_copy(xTb[:, :, m * P:(m + 1) * P], xchunk[:])

                pg = ps_g.tile([P, E], F32, space="PSUM")
                for k in range(KD):
                    nc.tensor.matmul(out=pg[:], lhsT=xchunk[:, k, :],
                                     rhs=gwt_sb[:, k, :],
                                     start=(k == 0), stop=False)
                # + gate bias via K=1 matmul with a ones row
                nc.tensor.matmul(out=pg[:], lhsT=onesf[:], rhs=gb_sb[:],
                                 start=False, stop=True)

                lg = gtmp.tile([P, E], F32)
                nc.scalar.activation(lg[:], pg[:], AF.Copy)
                m8 = gtmp.tile([P, 8], F32)
                nc.vector.max(m8[:], lg[:])
                # exp(l - max)
                ex = gtmp.tile([P, E], F32)
                nc.vector.tensor_scalar(out=ex[:], in0=lg[:],
                                        scalar1=m8[:, 0:1], scalar2=None,
                                        op0=OP.subtract)
                nc.scalar.activation(ex[:], ex[:], AF.Exp, bias=zerob[:])
                # top-2 mask
                mask = gtmp.tile([P, E], F32)
                nc.vector.tensor_scalar(out=mask[:], in0=lg[:],
                                        scalar1=m8[:, 1:2], scalar2=None,
                                        op0=OP.is_ge)
                # denom = 1 + exp(second - max);  cw = mask * ex / denom
                e2 = gtmp.tile([P, 1], F32)
                nc.vector.tensor_tensor(out=e2[:], in0=m8[:, 1:2], in1=m8[:, 0:1],
                                        op=OP.subtract)
                nc.scalar.activation(e2[:], e2[:], AF.Exp, bias=zerob[:])
                den = gtmp.tile([P, 1], F32)
                nc.vector.tensor_scalar(out=den[:], in0=e2[:], scalar1=1.0,
                                        scalar2=None, op0=OP.add)
                rec = gtmp.tile([P, 1], F32)
                nc.vector.reciprocal(rec[:], den[:])
                cwm = gtmp.tile([P, E], F32)
                nc.vector.tensor_mul(cwm[:], ex[:], mask[:])
                nc.vector.tensor_scalar(out=cw[:, m, 0:E], in0=cwm[:],
                                        scalar1=rec[:, 0:1], scalar2=None,
                                        op0=OP.mult)
                if NV > E:
                    nc.vector.memset(cw[:, m, E:NV], 1.0)

            # ---- virtual experts ----
            for e in range(NV):
                w2sb = w2s.tile([P, HCN, D], BF)
                nc.sync.dma_start(out=w2sb[:], in_=w2t[e])
                b1sb = b13.tile([P, HCN], F32)
                nc.sync.dma_start(out=b1sb[:], in_=b1a[e])
                b3sb = b13.tile([P, HCN], F32)
                nc.sync.dma_start(out=b3sb[:], in_=b3a[e])

                hT = hpool.tile([P, HCN, T], BF)

                # phase A: hT[h, t] = silu(W1 x + b1) * (W3 x + b3), feature-major
                for hc in range(HCN):
                    w1c = w1s.tile([P, KD, P], BF)
                    nc.sync.dma_start(out=w1c[:], in_=w1t[e, hc])
                    w3c = w1s.tile([P, KD, P], BF)
                    nc.sync.dma_start(out=w3c[:], in_=w3t[e, hc])
                    for ft in range(FT):
                        fsl = slice(ft * 512, min((ft + 1) * 512, T))
                        fw = fsl.stop - fsl.start
                        o1 = ps_l1.tile([P, 512], F32, space="PSUM", name="o1")
                        for k in range(KD):
                            nc.tensor.matmul(out=o1[:, :fw], lhsT=w1c[:, k, :],
                                             rhs=xTb[:, k, fsl],
                                             start=(k == 0), stop=(k == KD - 1))
                        # silu(v) = v * sigmoid(v), v = o1 + b1
                        s1 = s1p.tile([P, 512], F32)
                        nc.scalar.activation(s1[:, :fw], o1[:, :fw], AF.Sigmoid,
                                             bias=b1sb[:, hc:hc + 1])
                        t1 = s1p.tile([P, 512], F32)
                        nc.vector.scalar_tensor_tensor(
                            out=t1[:, :fw], in0=o1[:, :fw],
                            scalar=b1sb[:, hc:hc + 1], in1=s1[:, :fw],
                            op0=OP.add, op1=OP.mult)
                        o3 = ps_l1.tile([P, 512], F32, space="PSUM", name="o3")
                        for k in range(KD):
                            nc.tensor.matmul(out=o3[:, :fw], lhsT=w3c[:, k, :],
                                             rhs=xTb[:, k, fsl],
                                             start=(k == 0), stop=(k == KD - 1))
                        # h = (o3 + b3) * silu_out
                        nc.vector.scalar_tensor_tensor(
                            out=hT[:, hc, fsl], in0=o3[:, :fw],
                            scalar=b3sb[:, hc:hc + 1], in1=t1[:, :fw],
                            op0=OP.add, op1=OP.mult)

                # phase B: yacc[t, d] (+)= cw[t, e] * (hT^T @ W2^T + b2)
                for tt in range(TT):
                    tsl = slice(tt * P, (tt + 1) * P)
                    for dt in range(DT):
                        dsl = slice(dt * 512, min((dt + 1) * 512, D))
                        dw = dsl.stop - dsl.start
                        yp = ps_y.tile([P, 512], F32, space="PSUM", name="yp")
                        nc.tensor.matmul(out=yp[:, :dw], lhsT=ones_sb[:],
                                         rhs=b2r_sb[0:1, e, dsl],
                                         start=True, stop=False)
                        for hc in range(HCN):
                            nc.tensor.matmul(out=yp[:, :dw],
                                             lhsT=hT[:, hc, tsl],
                                             rhs=w2sb[:, hc, dsl],
                                             start=False, stop=(hc == HCN - 1))
                        if e == 0:
                            nc.vector.tensor_scalar(
                                out=yacc[:, tt, dsl], in0=yp[:, :dw],
                                scalar1=cw[:, tt, e:e + 1], scalar2=None,
                                op0=OP.mult)
                        else:
                            nc.vector.scalar_tensor_tensor(
                                out=yacc[:, tt, dsl], in0=yp[:, :dw],
                                scalar=cw[:, tt, e:e + 1],
                                in1=yacc[:, tt, dsl],
                                op0=OP.mult, op1=OP.add)

            nc.sync.dma_start(out=y[:], in_=yacc[:])

    nc.compile()
    return nc


# ---------------- host-side packing ----------------

def pack_static(cfg: Cfg, gate_w, gate_b, w1, b1, w2, b2, w3, b3,
                sw1, sb1, sw2, sb2, sw3, sb3):
    D, H, E, NV, n_sh = cfg.D, cfg.H, cfg.E, cfg.NV, cfg.n_sh
    KD, HCN = cfg.KD, cfg.HCN

    w1T = np.transpose(w1, (0, 2, 1))                      # [E, D, H]
    w3T = np.transpose(w3, (0, 2, 1))
    w2T = np.transpose(w2, (0, 2, 1))                      # [E, H, D]
    s1T = sw1.T.reshape(D, n_sh, H).transpose(1, 0, 2)     # [n_sh, D, H]
    s3T = sw3.T.reshape(D, n_sh, H).transpose(1, 0, 2)
    s2T = sw2.T.reshape(n_sh, H, D)                        # [n_sh, H, D]
    w1T_all = np.concatenate([w1T, s1T], 0)                # [NV, D, H]
    w3T_all = np.concatenate([w3T, s3T], 0)
    w2T_all = np.concatenate([w2T, s2T], 0)                # [NV, H, D]

    w1t = np.ascontiguousarray(
        w1T_all.reshape(NV, KD, P, HCN, P).transpose(0, 3, 2, 1, 4)).astype(BF16)
    w3t = np.ascontiguousarray(
        w3T_all.reshape(NV, KD, P, HCN, P).transpose(0, 3, 2, 1, 4)).astype(BF16)
    w2t = np.ascontiguousarray(
        w2T_all.reshape(NV, HCN, P, D).transpose(0, 2, 1, 3)).astype(BF16)

    b1_all = np.concatenate([b1, sb1.reshape(n_sh, H)], 0)  # [NV, H]
    b3_all = np.concatenate([b3, sb3.reshape(n_sh, H)], 0)
    b1a = np.ascontiguousarray(
        b1_all.reshape(NV, HCN, P).transpose(0, 2, 1)).astype(np.float32)
    b3a = np.ascontiguousarray(
        b3_all.reshape(NV, HCN, P).transpose(0, 2, 1)).astype(np.float32)

    b2_all = np.concatenate(
        [b2, sb2[None], np.zeros((n_sh - 1, D), np.float32)], 0)  # [NV, D]
    b2r = b2_all[None].astype(BF16)                         # [1, NV, D]

    gwt = np.ascontiguousarray(
        gate_w.T.reshape(KD, P, E).transpose(1, 0, 2)).astype(np.float32)
    gb = gate_b[None].astype(np.float32)
    ones1 = np.ones((1, P), BF16)

    return dict(w1t=w1t, w3t=w3t, w2t=w2t, b1a=b1a, b3a=b3a, b2r=b2r,
                gwt=gwt, gb=gb, ones1=ones1)


def pack_xT(cfg: Cfg, x_tokens):
    """x_tokens [T, D] fp32 -> xT device layout [P, KD, T]."""
    T, D = x_tokens.shape
    xT = x_tokens.T.reshape(cfg.KD, P, T).transpose(1, 0, 2)
    return np.ascontiguousarray(xT).astype(np.float32)


def unpack_y(cfg: Cfg, y_dev):
    """y device layout [P, TT, D] -> [T, D]."""
    return np.ascontiguousarray(y_dev.transpose(1, 0, 2).reshape(cfg.T, cfg.D))


_CACHE = {}


def _get_nc(cfg: Cfg):
    key = (cfg.D, cfg.H, cfg.E, cfg.n_sh, cfg.T)
    if key not in _CACHE:
        _CACHE[key] = build_nc(cfg)
    return _CACHE[key]


def make_in_maps(cfg: Cfg, inputs):
    static = pack_static(
        cfg,
        np.asarray(inputs["gate_w"], np.float32), np.asarray(inputs["gate_b"], np.float32),
        np.asarray(inputs["w1"], np.float32), np.asarray(inputs["b1"], np.float32),
        np.asarray(inputs["w2"], np.float32), np.asarray(inputs["b2"], np.float32),
        np.asarray(inputs["w3"], np.float32), np.asarray(inputs["b3"], np.float32),
        np.asarray(inputs["sw1"], np.float32), np.asarray(inputs["sb1"], np.float32),
        np.asarray(inputs["sw2"], np.float32), np.asarray(inputs["sb2"], np.float32),
        np.asarray(inputs["sw3"], np.float32), np.asarray(inputs["sb3"], np.float32),
    )
    x = np.asarray(inputs["x"], np.float32)
    B, S, D = x.shape
    xf = x.reshape(-1, D)
    in_maps = []
    for c in range(cfg.n_cores):
        m = dict(static)
        m["xT"] = pack_xT(cfg, xf[c * cfg.T:(c + 1) * cfg.T])
        in_maps.append(m)
    return in_maps


def kernel(**inputs) -> np.ndarray:
    x = np.asarray(inputs["x"], np.float32)
    B, S, D = x.shape
    N = B * S
    cfg = Cfg(D=D, T=N // 8, n_cores=8)
    nc = _get_nc(cfg)
    in_maps = make_in_maps(cfg, inputs)
    res = run_bass_kernel_spmd(nc, in_maps, list(range(cfg.n_cores)))
    outs = [unpack_y(cfg, res.results[c]["y"]) for c in range(cfg.n_cores)]
    return np.concatenate(outs, 0).reshape(B, S, D)
